# revision 4
# baseline (speedup 1.0000x reference)
"""AttentionHead kernel for Trainium2 (8 NeuronCores, data-parallel over batch).

Computes, per batch element:
  q = query @ Wq + bq ; k = key @ Wk + bk ; v = value @ Wv + bv
  qn = q / |q| ; kn = k / |k|
  out = softmax((qn @ kn^T) / sqrt(64)) @ v

Layout strategy (per core, one batch element):
  - Inputs [S=2048, DIN=768] are loaded in 128-token tiles and transposed
    on the PE (128x128 blocks) so features land on partitions.
  - Projections computed in transposed form: qT/kT/vT [64, S]
    (lhsT = W chunk [128f, 64], rhs = inputT chunk [128f, ntok]).
  - L2 normalization along features = partition direction: sum-of-squares
    via ones-vector matmul, rsqrt via DVE reciprocal + ACT sqrt, broadcast
    back over partitions via a K=1 matmul.
  - scoresT [keys, q] = knT_chunk^T @ qnT  (keys on partitions).
    Softmax needs no max-subtraction: scores are cosines/8 in [-1/8, 1/8].
  - exp on ACT with scale=1/8 fused; denominator comes free by augmenting
    v with a ones column: outT_aug [65, q] += v_aug[keys,65]^T @ expT.
  - Final: transpose [65,128] blocks back, divide by denominator, DMA out.
"""

import sys

sys.path.insert(0, "/opt/trn_rl_repo")

import os
import numpy as np

import concourse.bass as bass
import concourse.tile as tile
from concourse import bacc, mybir
from concourse.bass_utils import run_bass_kernel_spmd
from concourse.masks import make_identity

P = 128
S = 2048
DIN = 768
DO = 64
NT = S // P  # 16 token tiles
NF = DIN // P  # 6 feature chunks
QC = 512  # q-chunk width (one PSUM bank of fp32)
NQ = S // QC  # 4 q chunks
F32 = mybir.dt.float32

# matmul input dtype for the heavy matmuls (projection / scores / attn@v)
MM_DT = {
    "f32": mybir.dt.float32,
    "f32r": mybir.dt.float32r,
    "bf16": mybir.dt.bfloat16,
}[os.environ.get("KERNEL_MM_DT", "f32")]


def build_program():
    nc = bacc.Bacc("TRN2", target_bir_lowering=False, debug=False)

    q_d = nc.dram_tensor("query", [S, DIN], F32, kind="ExternalInput").ap()
    k_d = nc.dram_tensor("key", [S, DIN], F32, kind="ExternalInput").ap()
    v_d = nc.dram_tensor("value", [S, DIN], F32, kind="ExternalInput").ap()
    w_d = {
        "q": nc.dram_tensor("Wq", [DIN, DO], F32, kind="ExternalInput").ap(),
        "k": nc.dram_tensor("Wk", [DIN, DO], F32, kind="ExternalInput").ap(),
        "v": nc.dram_tensor("Wv", [DIN, DO], F32, kind="ExternalInput").ap(),
    }
    b_d = {
        "q": nc.dram_tensor("bq", [DO, 1], F32, kind="ExternalInput").ap(),
        "k": nc.dram_tensor("bk", [DO, 1], F32, kind="ExternalInput").ap(),
        "v": nc.dram_tensor("bv", [DO, 1], F32, kind="ExternalInput").ap(),
    }
    out_d = nc.dram_tensor("out", [S, DO], F32, kind="ExternalOutput").ap()

    with tile.TileContext(nc) as tc:
        with (
            tc.tile_pool(name="consts", bufs=1) as consts,
            tc.tile_pool(name="persist", bufs=1) as persist,
        ):
            ident = consts.tile([P, P], F32, name="ident", tag="ident")
            make_identity(nc, ident)
            ones_c = consts.tile([DO, 1], F32, name="ones_c", tag="ones_c")  # lhsT for colsum
            nc.vector.memset(ones_c, 1.0)
            ones_r = consts.tile([1, DO], F32, name="ones_r", tag="ones_r")  # lhsT for bcast
            nc.vector.memset(ones_r, 1.0)

            wt = {}
            bt = {}
            for t in ("q", "k", "v"):
                wt[t] = consts.tile([P, NF * DO], F32, name=f"w{t}", tag=f"w{t}")
                nc.sync.dma_start(
                    wt[t].rearrange("p (c o) -> p c o", c=NF),
                    w_d[t].rearrange("(c p) o -> p c o", p=P),
                )
                bt[t] = consts.tile([DO, 1], F32, name=f"b{t}", tag=f"b{t}")
                nc.sync.dma_start(bt[t][:], b_d[t])

            # persistent SBUF state
            qT = persist.tile([DO, S], F32, tag="qT")
            kT = persist.tile([DO, S], F32, tag="kT")
            qnT = persist.tile([DO, S], F32, tag="qnT")
            knT = persist.tile([DO, S], F32, tag="knT")
            vaug = persist.tile([P, NT * (DO + 1)], F32, name="vaug", tag="vaug")
            nc.vector.memset(vaug, 1.0)  # ones column; cols 0..63 overwritten
            sq = persist.tile([DO, S], F32, name="sq", tag="sq")
            rq = persist.tile([1, S], F32, name="rq", tag="rq")
            rk = persist.tile([1, S], F32, name="rk", tag="rk")
            tmprow = persist.tile([1, S], F32, name="tmprow", tag="tmprow")

            # ---------------- projections ----------------
            with (
                tc.tile_pool(name="inp", bufs=3) as inp,
                tc.tile_pool(name="tch", bufs=6) as tch,
                tc.tile_pool(name="pt", bufs=3, space="PSUM") as pt,
                tc.tile_pool(name="pproj", bufs=2, space="PSUM") as pproj,
                tc.tile_pool(name="pcol", bufs=1, space="PSUM") as pcol,
                tc.tile_pool(name="pbc", bufs=1, space="PSUM") as pbc,
                tc.tile_pool(name="vts", bufs=2) as vts_pool,
            ):

                def project_tile(src_ap, i, which):
                    """Load token tile i of src, return psum tile [64, 128]
                    holding (src_tile @ W).T before bias."""
                    it = inp.tile([P, DIN], F32, name="in", tag="in")
                    nc.sync.dma_start(it[:], src_ap[i * P : (i + 1) * P, :])
                    pp = pproj.tile([DO, P], F32, name="pp", tag="pp")
                    for c in range(NF):
                        tp = pt.tile([P, P], F32, name="tp", tag="tp")
                        nc.tensor.transpose(
                            tp[:], it[:, c * P : (c + 1) * P], ident[:]
                        )
                        tcs = tch.tile([P, P], F32, name="tcs", tag="tcs")
                        nc.any.tensor_copy(tcs[:], tp[:])
                        nc.tensor.matmul(
                            pp[:],
                            lhsT=wt[which][:, c * DO : (c + 1) * DO],
                            rhs=tcs[:],
                            start=(c == 0),
                            stop=(c == NF - 1),
                        )
                    return pp

                def normalize(srcT, dstT, r_row):
                    """dstT = srcT * (1/|col|); r_row gets 1/|col|."""
                    nc.vector.tensor_mul(sq[:], srcT[:], srcT[:])
                    for m in range(NQ):
                        cs = slice(m * QC, (m + 1) * QC)
                        pc = pcol.tile([1, QC], F32, name="pc", tag="pc")
                        nc.tensor.matmul(
                            pc[:], lhsT=ones_c[:], rhs=sq[:, cs],
                            start=True, stop=True,
                        )
                        nc.vector.reciprocal(tmprow[0:1, cs], pc[:])
                        nc.scalar.activation(
                            r_row[0:1, cs],
                            tmprow[0:1, cs],
                            mybir.ActivationFunctionType.Sqrt,
                        )
                        pb = pbc.tile([DO, QC], F32, name="pb", tag="pb")
                        nc.tensor.matmul(
                            pb[:], lhsT=ones_r[:], rhs=r_row[0:1, cs],
                            start=True, stop=True,
                        )
                        nc.vector.tensor_mul(dstT[:, cs], srcT[:, cs], pb[:])

                # query side
                for i in range(NT):
                    pp = project_tile(q_d, i, "q")
                    nc.vector.tensor_scalar_add(
                        qT[:, i * P : (i + 1) * P], pp[:], bt["q"][:]
                    )
                normalize(qT, qnT, rq)

                # key / value side
                for i in range(NT):
                    pp = project_tile(k_d, i, "k")
                    nc.vector.tensor_scalar_add(
                        kT[:, i * P : (i + 1) * P], pp[:], bt["k"][:]
                    )
                    pv = project_tile(v_d, i, "v")
                    vts = vts_pool.tile([DO, P], F32, name="vts", tag="vts")
                    nc.vector.tensor_scalar_add(vts[:], pv[:], bt["v"][:])
                    # transpose back to natural layout [128 tok, 64]
                    pvn = pt.tile([P, DO], F32, name="tp", tag="tp")
                    nc.tensor.transpose(pvn[:], vts[:], ident[0:DO, 0:DO])
                    nc.any.tensor_copy(
                        vaug[:, i * (DO + 1) : i * (DO + 1) + DO], pvn[:]
                    )
                normalize(kT, knT, rk)

            # ---------------- attention ----------------
            with (
                tc.tile_pool(name="psc", bufs=3, space="PSUM") as psc,
                tc.tile_pool(name="pout", bufs=2, space="PSUM") as pout,
                tc.tile_pool(name="pfin", bufs=2, space="PSUM") as pfin,
                tc.tile_pool(name="expb", bufs=4) as expb,
                tc.tile_pool(name="osb", bufs=2) as osb,
                tc.tile_pool(name="fin", bufs=4) as fin_pool,
            ):
                for j in range(NQ):
                    qs = slice(j * QC, (j + 1) * QC)
                    po = pout.tile([DO + 1, QC], F32, name="po", tag="po")
                    for c in range(NT):
                        ps = psc.tile([P, QC], F32, name="ps", tag="ps")
                        nc.tensor.matmul(
                            ps[:],
                            lhsT=knT[:, c * P : (c + 1) * P],
                            rhs=qnT[:, qs],
                            start=True,
                            stop=True,
                        )
                        et = expb.tile([P, QC], F32, name="et", tag="et")
                        nc.scalar.activation(
                            et[:],
                            ps[:],
                            mybir.ActivationFunctionType.Exp,
                            bias=0.0,
                            scale=0.125,
                        )
                        nc.tensor.matmul(
                            po[:],
                            lhsT=vaug[:, c * (DO + 1) : (c + 1) * (DO + 1)],
                            rhs=et[:],
                            start=(c == 0),
                            stop=(c == NT - 1),
                        )
                    ot = osb.tile([DO + 1, QC], F32, name="ot", tag="ot")
                    nc.any.tensor_copy(ot[:], po[:])
                    for m in range(QC // P):
                        pf = pfin.tile([P, DO + 1], F32, name="pf", tag="pf")
                        nc.tensor.transpose(
                            pf[:],
                            ot[:, m * P : (m + 1) * P],
                            ident[0 : DO + 1, 0 : DO + 1],
                        )
                        rec = fin_pool.tile([P, 1], F32, name="rec", tag="rec")
                        nc.vector.reciprocal(rec[:], pf[:, DO : DO + 1])
                        fin = fin_pool.tile([P, DO], F32, name="fin", tag="fin")
                        nc.vector.tensor_scalar_mul(fin[:], pf[:, 0:DO], rec[:])
                        row0 = (j * QC // P + m) * P
                        nc.sync.dma_start(out_d[row0 : row0 + P, :], fin[:])

    nc.compile()
    return nc


_CACHE = {}


def _get_program():
    if "nc" not in _CACHE:
        _CACHE["nc"] = build_program()
    return _CACHE["nc"]


def kernel(query, key, value, Wq, bq, Wk, bk, Wv, bv):
    nc = _get_program()
    query = np.asarray(query, np.float32)
    key = np.asarray(key, np.float32)
    value = np.asarray(value, np.float32)
    shared = {
        "Wq": np.ascontiguousarray(Wq, np.float32),
        "Wk": np.ascontiguousarray(Wk, np.float32),
        "Wv": np.ascontiguousarray(Wv, np.float32),
        "bq": np.ascontiguousarray(np.asarray(bq, np.float32).reshape(DO, 1)),
        "bk": np.ascontiguousarray(np.asarray(bk, np.float32).reshape(DO, 1)),
        "bv": np.ascontiguousarray(np.asarray(bv, np.float32).reshape(DO, 1)),
    }
    B = query.shape[0]
    assert B == 8, f"kernel hardcoded for B=8, got {B}"
    in_maps = [
        {
            "query": np.ascontiguousarray(query[b]),
            "key": np.ascontiguousarray(key[b]),
            "value": np.ascontiguousarray(value[b]),
            **shared,
        }
        for b in range(B)
    ]
    res = run_bass_kernel_spmd(nc, in_maps, list(range(B)))
    return np.stack([res.results[b]["out"] for b in range(B)], axis=0)


def _install_ntff_hook():
    """Provide antenv.axon_hooks + register the ctypes NTFF hook that
    trn_boot skips when the module is absent."""
    import types

    if "antenv.axon_hooks" not in sys.modules:
        mod = types.ModuleType("antenv.axon_hooks")
        state = {"hook": None}
        mod.set_axon_ntff_profile_hook = lambda h: state.__setitem__("hook", h)
        mod.get_axon_ntff_profile_hook = lambda: state["hook"]
        sys.modules["antenv.axon_hooks"] = mod
    mod = sys.modules["antenv.axon_hooks"]
    if mod.get_axon_ntff_profile_hook() is None:
        sys.path.insert(0, "/root/.axon_site/trn_agent_boot")
        import trn_boot

        hook = trn_boot._ntff_profile_via_ctypes("/opt/axon/libaxon_pjrt.so")
        mod.set_axon_ntff_profile_hook(hook)


def run_traced(inputs):
    """Like kernel() but with NTFF profiling; returns (out, exec_time_ns)."""
    _install_ntff_hook()
    nc = _get_program()
    query = np.asarray(inputs["query"], np.float32)
    key = np.asarray(inputs["key"], np.float32)
    value = np.asarray(inputs["value"], np.float32)
    shared = {
        "Wq": np.ascontiguousarray(inputs["Wq"], np.float32),
        "Wk": np.ascontiguousarray(inputs["Wk"], np.float32),
        "Wv": np.ascontiguousarray(inputs["Wv"], np.float32),
        "bq": np.ascontiguousarray(np.asarray(inputs["bq"], np.float32).reshape(DO, 1)),
        "bk": np.ascontiguousarray(np.asarray(inputs["bk"], np.float32).reshape(DO, 1)),
        "bv": np.ascontiguousarray(np.asarray(inputs["bv"], np.float32).reshape(DO, 1)),
    }
    B = query.shape[0]
    in_maps = [
        {
            "query": np.ascontiguousarray(query[b]),
            "key": np.ascontiguousarray(key[b]),
            "value": np.ascontiguousarray(value[b]),
            **shared,
        }
        for b in range(B)
    ]
    res = run_bass_kernel_spmd(nc, in_maps, list(range(B)), trace=True)
    out = np.stack([res.results[b]["out"] for b in range(B)], axis=0)
    return out, res.exec_time_ns


# revision 6
# speedup vs baseline: 1.5798x; 1.5798x over previous
"""AttentionHead kernel for Trainium2 (8 NeuronCores, data-parallel over batch).

Computes, per batch element:
  q = query @ Wq + bq ; k = key @ Wk + bk ; v = value @ Wv + bv
  qn = q / |q| ; kn = k / |k|
  out = softmax((qn @ kn^T) / sqrt(64)) @ v

Per-core design (one batch element per core):
  - Inputs [S=2048, DIN=768] stream in 128-token tiles, cast f32->bf16 during
    the DMA (SWDGE), then PE-transposed in 128x128 blocks so features land on
    partitions.
  - Projections in transposed form: qT/kT/vT [64, S]
    (lhsT = W chunk [128f, 64] bf16, rhs = inputT chunk [128f, 512t] bf16,
     fp32 PSUM accumulation over 6 feature chunks).
  - L2 norm along features (partition dim): ones-vector matmul for sum of
    squares, DVE reciprocal + ACT sqrt, K=1 matmul to broadcast across
    partitions, DVE multiply -> qnT/knT bf16.
  - scoresT [keys, q] = knT_chunk^T @ qnT; softmax needs no max-subtract
    (scores are cosines/8 in [-1/8, 1/8]); ACT exp with scale=1/8 fused,
    bf16 out. Denominator via ones-column in v_aug [128 keys, 65]:
    outT_aug [65, q] += v_aug^T @ expT  (fp32 PSUM).
  - k/v groups of 4 tiles stream; each group's attention contribution is
    accumulated right behind its projection so DMA and compute overlap.
  - Final: transpose [65,128] blocks, multiply by reciprocal denominator,
    DMA out fp32.
"""

import sys

sys.path.insert(0, "/opt/trn_rl_repo")

import os
import numpy as np

import concourse.bass as bass
import concourse.tile as tile
from concourse import bacc, mybir
from concourse.bass_utils import run_bass_kernel_spmd
from concourse.masks import make_identity

P = 128
S = 2048
DIN = 768
DO = 64
NT = S // P  # 16 token tiles
NF = DIN // P  # 6 feature chunks
G = 4  # token tiles per group
NG = NT // G  # 4 groups
GW = G * P  # 512 tokens per group
QC = 512  # q-chunk width for attention
NQ = S // QC
F32 = mybir.dt.float32
BF16 = mybir.dt.bfloat16

AF = mybir.ActivationFunctionType


def build_program():
    nc = bacc.Bacc("TRN2", target_bir_lowering=False, debug=False)

    src_d = {
        "q": nc.dram_tensor("query", [S, DIN], F32, kind="ExternalInput").ap(),
        "k": nc.dram_tensor("key", [S, DIN], F32, kind="ExternalInput").ap(),
        "v": nc.dram_tensor("value", [S, DIN], F32, kind="ExternalInput").ap(),
    }
    w_d = {
        "q": nc.dram_tensor("Wq", [DIN, DO], F32, kind="ExternalInput").ap(),
        "k": nc.dram_tensor("Wk", [DIN, DO], F32, kind="ExternalInput").ap(),
        "v": nc.dram_tensor("Wv", [DIN, DO], F32, kind="ExternalInput").ap(),
    }
    b_d = {
        "q": nc.dram_tensor("bq", [DO, 1], F32, kind="ExternalInput").ap(),
        "k": nc.dram_tensor("bk", [DO, 1], F32, kind="ExternalInput").ap(),
        "v": nc.dram_tensor("bv", [DO, 1], F32, kind="ExternalInput").ap(),
    }
    out_d = nc.dram_tensor("out", [S, DO], F32, kind="ExternalOutput").ap()

    with tile.TileContext(nc) as tc:
        with (
            tc.tile_pool(name="consts", bufs=1) as consts,
            tc.tile_pool(name="persist", bufs=1) as persist,
            tc.tile_pool(name="inp", bufs=2 * G + 2) as inp,
            tc.tile_pool(name="tq", bufs=1) as tqp,
            tc.tile_pool(name="expb", bufs=4) as expb,
            tc.tile_pool(name="fin", bufs=4) as fin_pool,
            # PSUM: ptr 2 + pproj 2 + pnorm 2 + psc 1 + pout 1 = 8 banks
            tc.tile_pool(name="ptr", bufs=2, space="PSUM") as ptr,
            tc.tile_pool(name="pproj", bufs=2, space="PSUM") as pproj,
            tc.tile_pool(name="pnorm", bufs=2, space="PSUM") as pnorm,
            tc.tile_pool(name="psc", bufs=1, space="PSUM") as psc,
            tc.tile_pool(name="pout", bufs=1, space="PSUM") as pout,
        ):
            identb = consts.tile([P, P], BF16, name="identb", tag="identb")
            make_identity(nc, identb)
            identf = consts.tile([DO + 1, DO + 1], F32, name="identf", tag="identf")
            make_identity(nc, identf)
            ones_c = consts.tile([DO, 1], F32, name="ones_c", tag="ones_c")
            nc.vector.memset(ones_c, 1.0)
            ones_r = consts.tile([1, DO], F32, name="ones_r", tag="ones_r")
            nc.vector.memset(ones_r, 1.0)

            wt = {}
            bt = {}
            for t in ("q", "k", "v"):
                wt[t] = consts.tile([P, NF * DO], BF16, name=f"w{t}", tag=f"w{t}")
                nc.gpsimd.dma_start(
                    wt[t].rearrange("p (c o) -> p c o", c=NF),
                    w_d[t].rearrange("(c p) o -> p c o", p=P),
                )
                bt[t] = consts.tile([DO, 1], F32, name=f"b{t}", tag=f"b{t}")
                nc.sync.dma_start(bt[t][:], b_d[t])

            # persistent SBUF state
            qnT = persist.tile([DO, S], BF16, name="qnT", tag="qnT")
            knT = persist.tile([DO, S], BF16, name="knT", tag="knT")
            vaug = persist.tile([P, NT * (DO + 1)], BF16, name="vaug", tag="vaug")
            nc.vector.memset(vaug, 1.0)
            xT = persist.tile([DO, GW], F32, name="xT", tag="xT")  # group scratch
            sq = persist.tile([DO, GW], F32, name="sq", tag="sq")
            rrow = persist.tile([1, GW], F32, name="rrow", tag="rrow")
            oacc = [
                persist.tile([DO + 1, QC], F32, name=f"oacc{j}", tag=f"oacc{j}")
                for j in range(NQ)
            ]

            # TQ[c]: transposed bf16 input chunks for one group,
            # [128 feat, 512 tok] each; reused across tensors/groups.
            TQ = [
                tqp.tile([P, GW], BF16, name=f"TQ{c}", tag=f"TQ{c}")
                for c in range(NF)
            ]

            def load_group(which, g):
                """DMA 4 token tiles (bf16 cast) -> list of sbuf tiles."""
                tiles = []
                for i in range(G):
                    it = inp.tile([P, DIN], BF16, name="in", tag="in")
                    r0 = (g * G + i) * P
                    nc.gpsimd.dma_start(it[:], src_d[which][r0 : r0 + P, :])
                    tiles.append(it)
                return tiles

            def transpose_group(tiles):
                """PE-transpose 4 tiles x 6 chunks into TQ[c][:, :512]."""
                for r in range(NF // 2):
                    for dc in range(2):
                        c = 2 * r + dc
                        tp = ptr.tile([P, GW], BF16, name="tp", tag="tp")
                        for i in range(G):
                            nc.tensor.transpose(
                                tp[:, i * P : (i + 1) * P],
                                tiles[i][:, c * P : (c + 1) * P],
                                identb[:],
                            )
                        nc.any.tensor_copy(TQ[c][:], tp[:])

            def project_group(which, g, dstT):
                """matmul TQ chunks with W -> dstT[64, 512] (f32 sbuf,
                bias added)."""
                pp = pproj.tile([DO, GW], F32, name="pp", tag="pp")
                for c in range(NF):
                    nc.tensor.matmul(
                        pp[:],
                        lhsT=wt[which][:, c * DO : (c + 1) * DO],
                        rhs=TQ[c][:],
                        start=(c == 0),
                        stop=(c == NF - 1),
                    )
                nc.vector.tensor_scalar_add(dstT[:], pp[:], bt[which][:])

            def normalize_group(srcT, dst_bf16_slice):
                """dst = src / |col| (cast to bf16)."""
                nc.vector.tensor_mul(sq[:], srcT[:], srcT[:])
                pc = pnorm.tile([1, GW], F32, name="pc", tag="pn")
                nc.tensor.matmul(
                    pc[:], lhsT=ones_c[:], rhs=sq[:], start=True, stop=True
                )
                nc.vector.reciprocal(rrow[:], pc[:])
                nc.scalar.activation(rrow[:], rrow[:], AF.Sqrt)
                pb = pnorm.tile([DO, GW], F32, name="pb", tag="pn")
                nc.tensor.matmul(
                    pb[:], lhsT=ones_r[:], rhs=rrow[:], start=True, stop=True
                )
                nc.vector.tensor_mul(dst_bf16_slice, srcT[:], pb[:])

            # ---------------- query side ----------------
            for g in range(NG):
                gs = slice(g * GW, (g + 1) * GW)
                tiles = load_group("q", g)
                transpose_group(tiles)
                project_group("q", g, xT)
                normalize_group(xT, qnT[:, gs])

            # ---------------- key/value stream + attention ----------------
            for g in range(NG):
                gs = slice(g * GW, (g + 1) * GW)
                # key group
                tiles = load_group("k", g)
                transpose_group(tiles)
                project_group("k", g, xT)
                normalize_group(xT, knT[:, gs])
                # value group
                tiles = load_group("v", g)
                transpose_group(tiles)
                project_group("v", g, xT)
                # v natural: transpose [64, 128] blocks back, into vaug
                vtb = fin_pool.tile([DO, GW], BF16, name="vtb", tag="vtb")
                nc.vector.tensor_copy(vtb[:], xT[:])
                for i in range(G):
                    ti = g * G + i
                    pvn = ptr.tile([P, GW], BF16, name="tp", tag="tp")
                    nc.tensor.transpose(
                        pvn[:, 0:DO],
                        vtb[:, i * P : (i + 1) * P],
                        identb[0:DO, 0:DO],
                    )
                    nc.any.tensor_copy(
                        vaug[:, ti * (DO + 1) : ti * (DO + 1) + DO], pvn[:, 0:DO]
                    )
                # attention contribution of this group's 4 key chunks
                for j in range(NQ):
                    qs = slice(j * QC, (j + 1) * QC)
                    po = pout.tile([DO + 1, QC], F32, name="po", tag="po")
                    for i in range(G):
                        c = g * G + i
                        ps = psc.tile([P, QC], F32, name="ps", tag="ps")
                        nc.tensor.matmul(
                            ps[:],
                            lhsT=knT[:, c * P : (c + 1) * P],
                            rhs=qnT[:, qs],
                            start=True,
                            stop=True,
                        )
                        et = expb.tile([P, QC], BF16, name="et", tag="et")
                        nc.scalar.activation(
                            et[:], ps[:], AF.Exp, bias=0.0, scale=0.125
                        )
                        nc.tensor.matmul(
                            po[:],
                            lhsT=vaug[:, c * (DO + 1) : (c + 1) * (DO + 1)],
                            rhs=et[:],
                            start=(i == 0),
                            stop=(i == G - 1),
                        )
                    if g == 0:
                        nc.vector.tensor_copy(oacc[j][:], po[:])
                    else:
                        nc.vector.tensor_add(oacc[j][:], oacc[j][:], po[:])

            # ---------------- finalize ----------------
            if True:
                for j in range(NQ):
                    for m in range(QC // P):
                        pf = pnorm.tile([P, DO + 1], F32, name="pf", tag="pn")
                        nc.tensor.transpose(
                            pf[:],
                            oacc[j][:, m * P : (m + 1) * P],
                            identf[:],
                        )
                        rec = fin_pool.tile([P, 1], F32, name="rec", tag="rec")
                        nc.vector.reciprocal(rec[:], pf[:, DO : DO + 1])
                        fin = fin_pool.tile([P, DO], F32, name="fin", tag="fin")
                        nc.vector.tensor_scalar_mul(fin[:], pf[:, 0:DO], rec[:])
                        row0 = (j * (QC // P) + m) * P
                        nc.sync.dma_start(out_d[row0 : row0 + P, :], fin[:])

    nc.compile()
    return nc


_CACHE = {}


def _get_program():
    if "nc" not in _CACHE:
        _CACHE["nc"] = build_program()
    return _CACHE["nc"]


def _make_in_maps(query, key, value, Wq, bq, Wk, bk, Wv, bv):
    query = np.asarray(query, np.float32)
    key = np.asarray(key, np.float32)
    value = np.asarray(value, np.float32)
    shared = {
        "Wq": np.ascontiguousarray(Wq, np.float32),
        "Wk": np.ascontiguousarray(Wk, np.float32),
        "Wv": np.ascontiguousarray(Wv, np.float32),
        "bq": np.ascontiguousarray(np.asarray(bq, np.float32).reshape(DO, 1)),
        "bk": np.ascontiguousarray(np.asarray(bk, np.float32).reshape(DO, 1)),
        "bv": np.ascontiguousarray(np.asarray(bv, np.float32).reshape(DO, 1)),
    }
    B = query.shape[0]
    assert B == 8, f"kernel hardcoded for B=8, got {B}"
    return [
        {
            "query": np.ascontiguousarray(query[b]),
            "key": np.ascontiguousarray(key[b]),
            "value": np.ascontiguousarray(value[b]),
            **shared,
        }
        for b in range(B)
    ]


def kernel(query, key, value, Wq, bq, Wk, bk, Wv, bv):
    nc = _get_program()
    in_maps = _make_in_maps(query, key, value, Wq, bq, Wk, bk, Wv, bv)
    res = run_bass_kernel_spmd(nc, in_maps, list(range(len(in_maps))))
    return np.stack([res.results[b]["out"] for b in range(len(in_maps))], axis=0)


def _install_ntff_hook():
    """Provide antenv.axon_hooks + register the ctypes NTFF hook that
    trn_boot skips when the module is absent."""
    import types

    if "antenv.axon_hooks" not in sys.modules:
        mod = types.ModuleType("antenv.axon_hooks")
        state = {"hook": None}
        mod.set_axon_ntff_profile_hook = lambda h: state.__setitem__("hook", h)
        mod.get_axon_ntff_profile_hook = lambda: state["hook"]
        sys.modules["antenv.axon_hooks"] = mod
    mod = sys.modules["antenv.axon_hooks"]
    if mod.get_axon_ntff_profile_hook() is None:
        sys.path.insert(0, "/root/.axon_site/trn_agent_boot")
        import trn_boot

        hook = trn_boot._ntff_profile_via_ctypes("/opt/axon/libaxon_pjrt.so")
        mod.set_axon_ntff_profile_hook(hook)


def run_traced(inputs):
    """Like kernel() but with NTFF profiling; returns (out, exec_time_ns)."""
    _install_ntff_hook()
    nc = _get_program()
    in_maps = _make_in_maps(
        inputs["query"], inputs["key"], inputs["value"],
        inputs["Wq"], inputs["bq"], inputs["Wk"], inputs["bk"],
        inputs["Wv"], inputs["bv"],
    )
    res = run_bass_kernel_spmd(nc, in_maps, list(range(len(in_maps))), trace=True)
    out = np.stack([res.results[b]["out"] for b in range(len(in_maps))], axis=0)
    return out, res.exec_time_ns
